# revision 31
# baseline (speedup 1.0000x reference)
"""CrossAttention kernel for 8 Trainium2 NeuronCores.

Data-parallel over batch: core b computes attention for tokens[b].
All device matmuls contract over the partition dim, so tokens are fed
pre-transposed ([hidden, T]) and scores/context vectors are kept in
transposed ([S, T] / [embed, T]) layout until the output projection,
which lands directly in [T, hidden] layout.

Q^T/K^T live in the UNPADDED 640-row layout (5 m-tiles, not 6): the
Q projection is 25 matmuls/chunk instead of 30. Per-head scores
segments must start on 32-aligned partitions, which head_dim=80 does
not give; instead K^T is kept in TWO parity-masked copies (kt_ev
zeroes odd heads' dims, kt_od zeroes even heads') so each head's
score matmul can extend its row range over the neighbouring head's
dims (which are zero in its copy) down to a legal 32-aligned base.
That also cuts scores to 12 matmuls/chunk (4 heads need 1
segment). One hardware quirk: a group-closing matmul of exactly 64
rows at partition base 0 faults, so head 3's closer extends to 80
rows (the extra 16 are the masked neighbour's, i.e. zeros).

Softmax (over S=77) runs in the partition dim: exp on ScalarE (no
max-subtraction needed: scores ~ N(0,1) in f32) into persistent
at-slot tiles whose rows 77:128 are zeroed ONCE at startup. The
denominator is a GPSIMD/Pool partition_all_reduce over the full 128
rows, reciprocal'd on DVE (reciprocal_approx_fast, SBUF only), and
the attention weights are normalized IN PLACE (bf16 x f32 -> bf16,
all-SBUF: 2x DVE mode) BEFORE attn@V. The attn@V PSUM is then
drained by plain copies, split between ScalarE and DVE for engine
balance. ctx keeps the padded 96-per-head row layout (the drain's
write bases must be 32-aligned), with each head's V columns rotated
by the head's ctx phase (96h mod 128) so read base == write base.

The bias bo rides for free: V column 80 of head 0 is ones, so the
normalized attn row sums to ~1.0 there, and the host stores bo in
(otherwise zero-padded) Wo row 80 - no bias add anywhere.

Software pipelining: scores run 3 head-slots ahead and the full
exp/all-reduce/recip/normalize chain 2 slots ahead of attn@V; chunk
c's head loop interleaves the output projection of chunk c-1 (odd
head slots) and the Q projection of chunk c+1 (head slots 0-4) so
the PE never waits on the softmax chain. Q/out projections share one
3-buf PSUM pool; PSUM = 3+3+2 banks.
"""

import numpy as np
import ml_dtypes

import concourse.bass as bass
import concourse.bacc as bacc
import concourse.bass_isa as bass_isa
import concourse.tile as tile
from concourse import mybir
import concourse.bass_utils as bass_utils

F32 = mybir.dt.float32
BF16 = mybir.dt.bfloat16

B, T, S = 8, 4096, 77
HID, EMB, CTX = 640, 640, 768
H, DH = 8, 80
DHP = 96            # ctx head stride (rows per head in ctx_v/wo layout)
EMBP = H * DHP      # 768 = 6 partition tiles of padded ctx rows
KT_H = HID // 128   # 5  k-tiles for hidden-contraction
KT_T = 2 * KT_H     # 10 k-tiles of stacked fp8 [t0..t3, r0..r3, t4, r4]
KT_W = 12           # wqs k-tiles: [w0..w4, w4, wl0..wl4, zeros]
F8 = mybir.dt.float8e4
WQ_SCALE = 32.0     # host scales Wq by this (keeps fp8 out of subnormals);
                    # un-scaled inside the exp's activation scale
KT_C = CTX // 128   # 6  k-tiles for ctx-contraction
MT_Q = EMB // 128   # 5  m-tiles of unpadded Q^T/K^T rows
MT_O = EMBP // 128  # 6  k-tiles of the output projection contraction
TCH = 512           # T chunk (one PSUM bank of f32)
NCH = T // TCH      # 8
P = 128
SCALE = 1.0 / np.sqrt(np.float32(DH))
NO1 = 512           # output projection column split (PSUM bank limit)
NSLOT = 4           # persistent at-slot ring depth
LS = 2              # scores lookahead (head slots)
LE = 1              # exp/denom chain lookahead

# Per-head scores segments (m, a, b) in the unpadded 640-row layout.
# Head h covers dims [80h, 80h+80); segments may extend over the
# adjacent head's dims because the parity-masked K^T zeroes them.
SC_SEGS = (
    ((0, 0, 80),),
    ((0, 64, 128), (1, 0, 32)),
    ((1, 0, 112),),
    ((1, 64, 128), (2, 0, 80)),
    ((2, 64, 128), (3, 0, 16)),
    ((3, 0, 96),),
    ((3, 96, 128), (4, 0, 48)),
    ((4, 0, 128),),
)

# Of the 14 ctx-drain pieces per chunk, indices in this set run on the
# scalar engine (Act), the rest on DVE - tuned for engine balance.
DRAIN_ACT = frozenset((0, 3, 7, 10))


def _part_cap(base):
    """Max partition count for an engine/PE access starting at `base`
    (within a 128-partition tile): base 0 -> 128, 64 -> 64, 32/96 -> 32."""
    b = base % P
    if b == 0:
        return P
    if b == 64:
        return 64
    assert b % 32 == 0, b
    return 32


def _matmul_segments(row0, nrows):
    """Split rows into (tile, a, b) pieces with legal partition base/count."""
    segs = []
    r = row0
    end = row0 + nrows
    while r < end:
        m, a = r // P, r % P
        c = min(end - r, _part_cap(a), P - a)
        segs.append((m, a, a + c))
        r += c
    return segs


def _build_program():
    nc = bacc.Bacc("TRN2", target_bir_lowering=False, debug=False, num_devices=B)

    tokT = nc.dram_tensor("tokT", [HID, T], BF16, kind="ExternalInput")
    ctxT = nc.dram_tensor("ctxT", [CTX, S], BF16, kind="ExternalInput")
    wq = nc.dram_tensor("wq", [HID, EMB], BF16, kind="ExternalInput")
    wke = nc.dram_tensor("wke", [CTX, EMB], BF16, kind="ExternalInput")
    wko = nc.dram_tensor("wko", [CTX, EMB], BF16, kind="ExternalInput")
    wv = nc.dram_tensor("wv", [CTX, EMB], BF16, kind="ExternalInput")
    wo = nc.dram_tensor("wo", [EMBP, HID], BF16, kind="ExternalInput")
    out = nc.dram_tensor("out", [T, HID], F32, kind="ExternalOutput")

    tokT_r = tokT.rearrange("(k p) t -> p k t", p=P)

    from contextlib import ExitStack
    with tile.TileContext(nc) as tc, ExitStack() as es:
        consts = es.enter_context(tc.tile_pool(name="consts", bufs=1))
        tok_pool = es.enter_context(tc.tile_pool(name="tok", bufs=3))
        qt_pool = es.enter_context(tc.tile_pool(name="qt", bufs=3))
        r_pool = es.enter_context(tc.tile_pool(name="r", bufs=4))
        ctxv_pool = es.enter_context(tc.tile_pool(name="ctxv", bufs=3))
        out_pool = es.enter_context(tc.tile_pool(name="outp", bufs=4))
        # PSUM: qproj/outproj share one 3-buf pool; 3 + 3 + 2 = 8 banks
        ps_qo = es.enter_context(tc.tile_pool(name="ps_qo", bufs=3, space="PSUM"))
        ps_s = es.enter_context(tc.tile_pool(name="ps_s", bufs=2, space="PSUM"))
        ps_cv = es.enter_context(tc.tile_pool(name="ps_cv", bufs=3, space="PSUM"))

        # ---- load weights / context (wq + first token chunks first so the
        # chunk-0 Q projection can start while the rest streams in) ----
        wq_r = wq.rearrange("(k p) n -> p k n", p=P)
        wq_sb = consts.tile([P, KT_H, EMB], BF16)
        toks = {}
        toks[0] = tok_pool.tile([P, KT_H, TCH], BF16, tag="tok", name="tok_sb")
        for k in range(KT_H):
            nc.sync.dma_start(out=wq_sb[:, k, :], in_=wq_r[:, k, :])
            # tok0 on the scalar engine's DMA queue: runs in parallel
            # with the wq stream on sync's queue
            nc.scalar.dma_start(out=toks[0][:, k, :],
                                in_=tokT_r[:, k, 0:TCH])
        wke_sb = consts.tile([P, KT_C, EMB], BF16)
        nc.sync.dma_start(out=wke_sb, in_=wke.rearrange("(k p) n -> p k n", p=P))
        ctx_sb = consts.tile([P, KT_C, S], BF16)
        nc.sync.dma_start(out=ctx_sb, in_=ctxT.rearrange("(k p) s -> p k s", p=P))
        wko_sb = consts.tile([P, KT_C, EMB], BF16)
        nc.sync.dma_start(out=wko_sb, in_=wko.rearrange("(k p) n -> p k n", p=P))
        wv_sb = consts.tile([P, KT_C, EMB], BF16)
        nc.sync.dma_start(out=wv_sb, in_=wv.rearrange("(k p) n -> p k n", p=P))
        toks[1] = tok_pool.tile([P, KT_H, TCH], BF16, tag="tok", name="tok_sb")
        nc.sync.dma_start(out=toks[1], in_=tokT_r[:, :, TCH:2 * TCH])
        wo_sb = consts.tile([P, MT_O, HID], BF16)
        nc.sync.dma_start(out=wo_sb, in_=wo.rearrange("(k p) n -> p k n", p=P))

        # persistent at slots; rows 77:128 zeroed once here (exp only ever
        # writes rows 0:77, so the pad rows stay zero for the all-reduce)
        at_sb = consts.tile([P, NSLOT, TCH], BF16, name="at_sb")
        nc.gpsimd.memset(at_sb[64:P, :, :], 0.0)

        # ---- Q projection m-tile: 8 fp8 DoubleRow matmuls (each contracts
        # a PAIR of k-tiles at 2x rate) + PSUM->SBUF copy on Act.
        # Terms: (tok8 + r8) @ wq8  (5 pairs over the stacked tokens), then
        # tok8 @ wq_lo8 (3 pairs; the last pairs r8_k0 against a zero
        # weight tile). Compensated fp8: quantization error ~0.1%. ----
        qts = {}

        def emit_qproj_mtile(c, m):
            if m == 0:
                qts[c] = qt_pool.tile([P, MT_Q, TCH], BF16, tag="qt",
                                      name="qt_sb")
            ps_q = ps_qo.tile([P, TCH], F32, tag="ps_qo", name="ps_q")
            mc = slice(m * P, (m + 1) * P)
            for k in range(KT_H):
                nc.tensor.matmul(
                    ps_q, wq_sb[:, k, mc],
                    toks[c][:, k, :], start=(k == 0), stop=(k == KT_H - 1))
            nc.scalar.copy(qts[c][:, m, :], ps_q)

        # ---- output projection subtile: 12 PE matmuls, PSUM->SBUF drains
        # split across Act/DVE (no bias add: bo rides Wo row 80), 1 DMA ----
        def emit_outproj_st(ctx_v, c, st, tail=False):
            tok_cols = ctx_v[:, :, st * P:(st + 1) * P]
            po1 = ps_qo.tile([P, NO1], F32, tag="ps_qo", name="po1")
            for k in range(MT_O):
                nc.tensor.matmul(po1, tok_cols[:, k, :], wo_sb[:, k, 0:NO1],
                                 start=(k == 0), stop=(k == MT_O - 1))
            po2 = ps_qo.tile([P, HID - NO1], F32, tag="ps_qo", name="po2")
            for k in range(MT_O):
                nc.tensor.matmul(po2, tok_cols[:, k, :], wo_sb[:, k, NO1:HID],
                                 start=(k == 0), stop=(k == MT_O - 1))
            out_sb = out_pool.tile([P, HID], F32)
            nc.scalar.copy(out_sb[:, 0:NO1], po1)
            # in the drain tail run the two PSUM copies on different
            # engines so they overlap
            (nc.vector.tensor_copy if tail else nc.scalar.copy)(
                out_sb[:, NO1:HID], po2)
            t0 = c * TCH + st * P
            nc.sync.dma_start(out=out[t0:t0 + P, :], in_=out_sb)

        # ---- K^T [EMB, S] as [128, 5, S], one parity-masked copy each ----
        # Emitted after the chunk-0 Q projection in PE order; only needs
        # wke/wko/ctx which stream in behind wq/tok0.
        def emit_kt(wk_sb, name):
            kt = consts.tile([P, MT_Q, S], BF16, name=name)
            for m in range(MT_Q):
                ps_k = ps_s.tile([P, S], F32, tag="ps_s", name="ps_k")
                for k in range(KT_C):
                    nc.tensor.matmul(
                        ps_k, wk_sb[:, k, m * P:(m + 1) * P], ctx_sb[:, k, :],
                        start=(k == 0), stop=(k == KT_C - 1))
                nc.vector.tensor_copy(kt[:, m, :], ps_k)
            return kt

        # ---- V [S, H, 128], with each head's columns ROTATED by the
        # head's ctx_v phase phi_h = (96h mod 128): V dim d sits at column
        # (phi_h + d) % 128. The attn@V output row (96h+off) % 128 then
        # equals ctx_v row 96h+off, so every drain piece has read base ==
        # write base (14 pieces per chunk). Head 0's col 80 is ones: the
        # normalized attn sums to ~1 there, which multiplies Wo row 80 =
        # bo (host-folded bias).
        def emit_v():
            v = consts.tile([S, H, P], BF16, name="v_sb")
            nc.vector.memset(v, 0.0)
            nc.vector.memset(v[:, 0, DH:DH + 1], 1.0)
            # all heads' V in two wide matmul groups (N=512/128) instead
            # of 8 narrow ones - 12 PE ops instead of 48 on the startup
            # critical path; copies scatter each head's rotated pieces
            for (n0, n1) in ((0, NO1), (NO1, EMB)):
                ps_v = ps_s.tile([S, n1 - n0], F32, tag="ps_s", name="ps_v")
                for k in range(KT_C):
                    nc.tensor.matmul(ps_v, ctx_sb[:, k, :],
                                     wv_sb[:, k, n0:n1],
                                     start=(k == 0), stop=(k == KT_C - 1))
                for h in range(H):
                    phi = (DHP * h) % P
                    base = h * DH
                    wrap = P - phi  # dims [0,wrap) at col phi+d, rest at d-wrap
                    for (d0, d1) in ((0, min(wrap, DH)), (min(wrap, DH), DH)):
                        g0 = max(base + d0, n0)
                        g1 = min(base + d1, n1)
                        if g0 >= g1:
                            continue
                        dd = g0 - base
                        col = phi + dd if dd < wrap else dd - wrap
                        nc.vector.tensor_copy(
                            v[:, h, col:col + (g1 - g0)],
                            ps_v[:, g0 - n0:g1 - n0])
            return v

        def emit_scores(c, h):
            kt = kt_ev if h % 2 == 0 else kt_od
            segs = SC_SEGS[h]
            ps_sc = ps_s.tile([S, TCH], F32, tag="ps_s", name="ps_sc")
            for i, (m, a, b) in enumerate(segs):
                nc.tensor.matmul(
                    ps_sc, kt[a:b, m, :], qts[c][a:b, m, :],
                    start=(i == 0), stop=(i == len(segs) - 1),
                    tile_position=(a, 0))
            return ps_sc

        # ---- chunk-0 prologue (interleaved with the DMA arrival order:
        # wqs/tok0, wke/ctx, tok1, wko, wv) ----
        for m in range(MT_Q):
            emit_qproj_mtile(0, m)
        kt_ev = emit_kt(wke_sb, "kt_ev")
        kt_od = emit_kt(wko_sb, "kt_od")
        v_sb = emit_v()

        prev_ctx = None  # ctx_v of the previous chunk (outproj deferred)
        for c in range(NCH):
            if c + 2 < NCH:
                toks[c + 2] = tok_pool.tile([P, KT_H, TCH], BF16, tag="tok",
                                            name="tok_sb")
                nc.sync.dma_start(
                    out=toks[c + 2],
                    in_=tokT_r[:, :, (c + 2) * TCH:(c + 3) * TCH])

            # exp into the head's at slot, then the denominator via Pool
            # all-reduce (SBUF only), reciprocal on DVE, and an IN-PLACE
            # bf16 normalize of the attention weights (so attn@V consumes
            # already-normalized weights and the PSUM drain is a plain copy)
            def emit_chain(c, h, ps_sc):
                slot = h % NSLOT
                nc.scalar.activation(
                    at_sb[0:S, slot, :], ps_sc,
                    mybir.ActivationFunctionType.Exp,
                    scale=float(SCALE))
                ar_sb = r_pool.tile([P, TCH], F32, tag="ar", name="ar_sb")
                nc.gpsimd.partition_all_reduce(
                    ar_sb, at_sb[:, slot, :], channels=P,
                    reduce_op=bass_isa.ReduceOp.add)
                rb_sb = r_pool.tile([P, TCH], F32, tag="rb", name="rb_sb")
                # full 128 partitions: the all-reduce broadcast the sum to
                # every partition, so no zero-divide; DVE ops keep the
                # 32-multiple partition counts the hardware likes
                nc.vector.reciprocal_approx_fast(out=rb_sb, in_=ar_sb)
                ats[h] = rb_sb

            # scores run LS slots ahead, the softmax chain LE slots ahead
            # of attn@V so the PE never blocks on the chain
            le = LE
            score_ps = {}
            ats = {}
            for j in range(min(LS, H)):
                score_ps[j] = emit_scores(c, j)
            for j in range(min(le, H)):
                emit_chain(c, j, score_ps.pop(j))
            ctx_v = ctxv_pool.tile([P, MT_O, TCH], BF16)
            epi = {}
            drain_i = 0
            for h in range(H):
                if h + LS < H:
                    score_ps[h + LS] = emit_scores(c, h + LS)
                if h + le < H:
                    emit_chain(c, h + le, score_ps.pop(h + le))

                # interleaved deferred outproj + next-chunk Q projection:
                # keeps the PE busy while the softmax chain runs ahead.
                # The LAST chunk's Q projection is split: m0-m2 during the
                # previous chunk (feeding the score lookahead), m3-m4 in
                # its own early slots - evens PE load so neither phase is
                # purely chain-paced.
                if c == NCH - 1:
                    if 1 <= h <= 4:
                        emit_outproj_st(prev_ctx, c - 1, h - 1)
                    elif h >= 6:
                        # open epilogue subtile (h-6): accumulate the k-tiles
                        # whose ctx rows are already drained (k0..k3)
                        st = h - 6
                        tok_cols = ctx_v[:, :, st * P:(st + 1) * P]
                        po1 = ps_qo.tile([P, NO1], F32, tag="ps_qo",
                                         name="po1")
                        for k in range(4):
                            nc.tensor.matmul(
                                po1, tok_cols[:, k, :], wo_sb[:, k, 0:NO1],
                                start=(k == 0), stop=False,
                                skip_group_check=True)
                        epi[st] = po1
                elif h % 2 == 1 and prev_ctx is not None:
                    emit_outproj_st(prev_ctx, c - 1, (h - 1) // 2)
                # chunk 0 additionally pulls qproj(2) m0-m2 into its empty
                # late slots (it has no outproj filler), chunk 1 emits the
                # remainder - same load-evening as at the tail
                if c == 0 and NCH > 2 and h >= 5:
                    emit_qproj_mtile(2, h - 5)
                if c + 1 < NCH:
                    if c == 1 and NCH > 2:
                        if h < 2:
                            emit_qproj_mtile(2, 3 + h)
                        elif False:
                            pass
                    else:
                        mlim = 3 if c + 1 == NCH - 1 else MT_Q
                        if h < mlim:
                            emit_qproj_mtile(c + 1, h)
                if c + 1 == NCH and h < MT_Q - 3:
                    emit_qproj_mtile(c, 3 + h)

                # ctx_aug^T [128, TCH] in the head's rotated row phase;
                # the drain normalizes on DVE (read base == write base
                # thanks to the V rotation; rb was computed off-chain)
                rb_sb = ats.pop(h)
                ps_c = ps_cv.tile([P, TCH], F32, tag="ps_cv", name="ps_c")
                nc.tensor.matmul(ps_c, v_sb[:, h, :],
                                 at_sb[0:S, h % NSLOT, :],
                                 start=True, stop=True)
                for (m, a, b) in _matmul_segments(h * DHP, DHP):
                    nc.vector.tensor_mul(
                        ctx_v[a:b, m, :], ps_c[a:b, :], rb_sb[a:b, :])
                    drain_i += 1

            prev_ctx = ctx_v

        # epilogue: close the two opened subtiles (k4/k5 + po2), then the
        # remaining two subtiles in full
        for st in range(2):
            po1 = epi[st]
            tok_cols = prev_ctx[:, :, st * P:(st + 1) * P]
            for k in (4, 5):
                nc.tensor.matmul(po1, tok_cols[:, k, :], wo_sb[:, k, 0:NO1],
                                 start=False, stop=(k == 5),
                                 skip_group_check=True)
            po2 = ps_qo.tile([P, HID - NO1], F32, tag="ps_qo", name="po2")
            for k in range(MT_O):
                nc.tensor.matmul(po2, tok_cols[:, k, :], wo_sb[:, k, NO1:HID],
                                 start=(k == 0), stop=(k == MT_O - 1))
            out_sb = out_pool.tile([P, HID], F32)
            nc.scalar.copy(out_sb[:, 0:NO1], po1)
            nc.vector.tensor_copy(out_sb[:, NO1:HID], po2)
            t0 = (NCH - 1) * TCH + st * P
            nc.sync.dma_start(out=out[t0:t0 + P, :], in_=out_sb)
        for st in range(2, TCH // P):
            emit_outproj_st(prev_ctx, NCH - 1, st, tail=True)

    nc.compile()
    return nc


_PROGRAM = None


def _get_program():
    global _PROGRAM
    if _PROGRAM is None:
        _PROGRAM = _build_program()
    return _PROGRAM


BF16_NP = ml_dtypes.bfloat16


def _pad_head_rows(w, dtype=np.float32):
    """[H*DH, cols] -> [H*DHP, cols] zero-padded per head."""
    wp = np.zeros((EMBP, w.shape[1]), dtype)
    for h in range(H):
        wp[h * DHP:h * DHP + DH] = w[h * DH:(h + 1) * DH]
    return wp


def _parity_mask(w, parity):
    """Zero the columns of heads whose index parity != parity."""
    wm = np.array(w, np.float32, copy=True)
    for h in range(H):
        if h % 2 != parity:
            wm[:, h * DH:(h + 1) * DH] = 0.0
    return wm


F8_NP = ml_dtypes.float8_e4m3


def _fp8_stack_tokens(tokT):
    """[640, T] f32 -> [1280, T] fp8: [t0..t3, r0..r3, t4, r4] k-tiles
    where t = fp8(x) and r = fp8(x - fp8(x))."""
    t8 = tokT.astype(F8_NP)
    r8 = (tokT - t8.astype(np.float32)).astype(F8_NP)
    return np.concatenate([t8[0:512], r8[0:512], t8[512:640], r8[512:640]],
                          axis=0)


def _fp8_stack_wq(Wq):
    """[640, 640] f32 -> [1536, 640] fp8 k-stack [w0..w4, w4, wl0..wl4, Z]
    for the 3-term Qproj (w = fp8(32 Wq), wl = fp8 residual)."""
    Wp = WQ_SCALE_NP * Wq
    w8 = Wp.astype(F8_NP)
    wlo = (Wp - w8.astype(np.float32)).astype(F8_NP)
    z = np.zeros((P, Wq.shape[1]), F8_NP)
    return np.concatenate([w8, w8[512:640], wlo, z], axis=0)


WQ_SCALE_NP = np.float32(32.0)


def _prepare_in_maps(tokens, context, Wq, Wk, Wv, Wo, bo):
    tokens = np.asarray(tokens, np.float32)
    context = np.asarray(context, np.float32)
    wq_ = np.ascontiguousarray(np.asarray(Wq, np.float32)).astype(BF16_NP)
    wk_f = np.asarray(Wk, np.float32)
    wke_ = _parity_mask(wk_f, 0).astype(BF16_NP)
    wko_ = _parity_mask(wk_f, 1).astype(BF16_NP)
    wv_ = np.ascontiguousarray(np.asarray(Wv, np.float32)).astype(BF16_NP)
    wo_ = _pad_head_rows(np.asarray(Wo, np.float32))
    wo_[DH] = np.asarray(bo, np.float32)   # bias rides Wo pad row 80
    wo_ = wo_.astype(BF16_NP)
    in_maps = []
    for b in range(B):
        in_maps.append({
            "tokT": np.ascontiguousarray(tokens[b].T).astype(BF16_NP),
            "ctxT": np.ascontiguousarray(context[b].T).astype(BF16_NP),
            "wq": wq_, "wke": wke_, "wko": wko_, "wv": wv_, "wo": wo_,
        })
    return in_maps


def kernel(tokens, context, Wq, Wk, Wv, Wo, bo):
    nc = _get_program()
    in_maps = _prepare_in_maps(tokens, context, Wq, Wk, Wv, Wo, bo)
    res = bass_utils.run_bass_kernel_spmd(nc, in_maps, core_ids=list(range(B)))
    return np.stack([res.results[b]["out"] for b in range(B)])


# revision 32
# speedup vs baseline: 1.0042x; 1.0042x over previous
"""CrossAttention kernel for 8 Trainium2 NeuronCores.

Data-parallel over batch: core b computes attention for tokens[b].
All device matmuls contract over the partition dim, so tokens are fed
pre-transposed ([hidden, T]) and scores/context vectors are kept in
transposed ([S, T] / [embed, T]) layout until the output projection,
which lands directly in [T, hidden] layout.

Q^T/K^T live in the UNPADDED 640-row layout (5 m-tiles, not 6): the
Q projection is 25 matmuls/chunk instead of 30. Per-head scores
segments must start on 32-aligned partitions, which head_dim=80 does
not give; instead K^T is kept in TWO parity-masked copies (kt_ev
zeroes odd heads' dims, kt_od zeroes even heads') so each head's
score matmul can extend its row range over the neighbouring head's
dims (which are zero in its copy) down to a legal 32-aligned base.
That also cuts scores to 12 matmuls/chunk (4 heads need 1
segment). One hardware quirk: a group-closing matmul of exactly 64
rows at partition base 0 faults, so head 3's closer extends to 80
rows (the extra 16 are the masked neighbour's, i.e. zeros).

Softmax (over S=77) runs in the partition dim: exp on ScalarE (no
max-subtraction needed: scores ~ N(0,1) in f32) into persistent
at-slot tiles whose rows 77:128 are zeroed ONCE at startup. The
denominator is a GPSIMD/Pool partition_all_reduce over the full 128
rows, reciprocal'd on DVE (reciprocal_approx_fast, SBUF only), and
the attention weights are normalized IN PLACE (bf16 x f32 -> bf16,
all-SBUF: 2x DVE mode) BEFORE attn@V. The attn@V PSUM is then
drained by plain copies, split between ScalarE and DVE for engine
balance. ctx keeps the padded 96-per-head row layout (the drain's
write bases must be 32-aligned), with each head's V columns rotated
by the head's ctx phase (96h mod 128) so read base == write base.

The bias bo rides for free: V column 80 of head 0 is ones, so the
normalized attn row sums to ~1.0 there, and the host stores bo in
(otherwise zero-padded) Wo row 80 - no bias add anywhere.

Software pipelining: scores run 3 head-slots ahead and the full
exp/all-reduce/recip/normalize chain 2 slots ahead of attn@V; chunk
c's head loop interleaves the output projection of chunk c-1 (odd
head slots) and the Q projection of chunk c+1 (head slots 0-4) so
the PE never waits on the softmax chain. Q/out projections share one
3-buf PSUM pool; PSUM = 3+3+2 banks.
"""

import numpy as np
import ml_dtypes

import concourse.bass as bass
import concourse.bacc as bacc
import concourse.bass_isa as bass_isa
import concourse.tile as tile
from concourse import mybir
import concourse.bass_utils as bass_utils

F32 = mybir.dt.float32
BF16 = mybir.dt.bfloat16

B, T, S = 8, 4096, 77
HID, EMB, CTX = 640, 640, 768
H, DH = 8, 80
DHP = 96            # ctx head stride (rows per head in ctx_v/wo layout)
EMBP = H * DHP      # 768 = 6 partition tiles of padded ctx rows
KT_H = HID // 128   # 5  k-tiles for hidden-contraction
KT_T = 2 * KT_H     # 10 k-tiles of stacked fp8 [t0..t3, r0..r3, t4, r4]
KT_W = 12           # wqs k-tiles: [w0..w4, w4, wl0..wl4, zeros]
F8 = mybir.dt.float8e4
WQ_SCALE = 32.0     # host scales Wq by this (keeps fp8 out of subnormals);
                    # un-scaled inside the exp's activation scale
KT_C = CTX // 128   # 6  k-tiles for ctx-contraction
MT_Q = EMB // 128   # 5  m-tiles of unpadded Q^T/K^T rows
MT_O = EMBP // 128  # 6  k-tiles of the output projection contraction
TCH = 512           # T chunk (one PSUM bank of f32)
NCH = T // TCH      # 8
P = 128
SCALE = 1.0 / np.sqrt(np.float32(DH))
NO1 = 512           # output projection column split (PSUM bank limit)
NSLOT = 4           # persistent at-slot ring depth
LS = 2              # scores lookahead (head slots)
LE = 1              # exp/denom chain lookahead

# Per-head scores segments (m, a, b) in the unpadded 640-row layout.
# Head h covers dims [80h, 80h+80); segments may extend over the
# adjacent head's dims because the parity-masked K^T zeroes them.
SC_SEGS = (
    ((0, 0, 80),),
    ((0, 64, 128), (1, 0, 32)),
    ((1, 0, 112),),
    ((1, 64, 128), (2, 0, 80)),
    ((2, 64, 128), (3, 0, 16)),
    ((3, 0, 96),),
    ((3, 96, 128), (4, 0, 48)),
    ((4, 0, 128),),
)

# Of the 14 ctx-drain pieces per chunk, indices in this set run on the
# scalar engine (Act), the rest on DVE - tuned for engine balance.
DRAIN_ACT = frozenset((0, 3, 7, 10))


def _part_cap(base):
    """Max partition count for an engine/PE access starting at `base`
    (within a 128-partition tile): base 0 -> 128, 64 -> 64, 32/96 -> 32."""
    b = base % P
    if b == 0:
        return P
    if b == 64:
        return 64
    assert b % 32 == 0, b
    return 32


def _matmul_segments(row0, nrows):
    """Split rows into (tile, a, b) pieces with legal partition base/count."""
    segs = []
    r = row0
    end = row0 + nrows
    while r < end:
        m, a = r // P, r % P
        c = min(end - r, _part_cap(a), P - a)
        segs.append((m, a, a + c))
        r += c
    return segs


def _build_program():
    nc = bacc.Bacc("TRN2", target_bir_lowering=False, debug=False, num_devices=B)

    tokT = nc.dram_tensor("tokT", [HID, T], BF16, kind="ExternalInput")
    ctxT = nc.dram_tensor("ctxT", [CTX, S], BF16, kind="ExternalInput")
    wq = nc.dram_tensor("wq", [HID, EMB], BF16, kind="ExternalInput")
    wke = nc.dram_tensor("wke", [CTX, EMB], BF16, kind="ExternalInput")
    wko = nc.dram_tensor("wko", [CTX, EMB], BF16, kind="ExternalInput")
    wv = nc.dram_tensor("wv", [CTX, EMB], BF16, kind="ExternalInput")
    wo = nc.dram_tensor("wo", [EMBP, HID], BF16, kind="ExternalInput")
    out = nc.dram_tensor("out", [T, HID], F32, kind="ExternalOutput")

    tokT_r = tokT.rearrange("(k p) t -> p k t", p=P)

    from contextlib import ExitStack
    with tile.TileContext(nc) as tc, ExitStack() as es:
        consts = es.enter_context(tc.tile_pool(name="consts", bufs=1))
        tok_pool = es.enter_context(tc.tile_pool(name="tok", bufs=3))
        qt_pool = es.enter_context(tc.tile_pool(name="qt", bufs=3))
        r_pool = es.enter_context(tc.tile_pool(name="r", bufs=4))
        ctxv_pool = es.enter_context(tc.tile_pool(name="ctxv", bufs=3))
        out_pool = es.enter_context(tc.tile_pool(name="outp", bufs=4))
        # PSUM: qproj/outproj share one 3-buf pool; 3 + 3 + 2 = 8 banks
        ps_qo = es.enter_context(tc.tile_pool(name="ps_qo", bufs=3, space="PSUM"))
        ps_s = es.enter_context(tc.tile_pool(name="ps_s", bufs=2, space="PSUM"))
        ps_cv = es.enter_context(tc.tile_pool(name="ps_cv", bufs=3, space="PSUM"))

        # ---- load weights / context (wq + first token chunks first so the
        # chunk-0 Q projection can start while the rest streams in) ----
        wq_r = wq.rearrange("(k p) n -> p k n", p=P)
        wq_sb = consts.tile([P, KT_H, EMB], BF16)
        toks = {}
        toks[0] = tok_pool.tile([P, KT_H, TCH], BF16, tag="tok", name="tok_sb")
        for k in range(KT_H):
            nc.sync.dma_start(out=wq_sb[:, k, :], in_=wq_r[:, k, :])
            # tok0 on the scalar engine's DMA queue: runs in parallel
            # with the wq stream on sync's queue
            nc.scalar.dma_start(out=toks[0][:, k, :],
                                in_=tokT_r[:, k, 0:TCH])
        wke_sb = consts.tile([P, KT_C, EMB], BF16)
        nc.sync.dma_start(out=wke_sb, in_=wke.rearrange("(k p) n -> p k n", p=P))
        ctx_sb = consts.tile([P, KT_C, S], BF16)
        nc.sync.dma_start(out=ctx_sb, in_=ctxT.rearrange("(k p) s -> p k s", p=P))
        wko_sb = consts.tile([P, KT_C, EMB], BF16)
        nc.sync.dma_start(out=wko_sb, in_=wko.rearrange("(k p) n -> p k n", p=P))
        wv_sb = consts.tile([P, KT_C, EMB], BF16)
        nc.sync.dma_start(out=wv_sb, in_=wv.rearrange("(k p) n -> p k n", p=P))
        toks[1] = tok_pool.tile([P, KT_H, TCH], BF16, tag="tok", name="tok_sb")
        nc.sync.dma_start(out=toks[1], in_=tokT_r[:, :, TCH:2 * TCH])
        wo_sb = consts.tile([P, MT_O, HID], BF16)
        nc.sync.dma_start(out=wo_sb, in_=wo.rearrange("(k p) n -> p k n", p=P))

        # persistent at slots; rows 77:128 zeroed once here (exp only ever
        # writes rows 0:77, so the pad rows stay zero for the all-reduce)
        at_sb = consts.tile([P, NSLOT, TCH], BF16, name="at_sb")
        nc.gpsimd.memset(at_sb[64:P, :, :], 0.0)

        # ---- Q projection m-tile: 8 fp8 DoubleRow matmuls (each contracts
        # a PAIR of k-tiles at 2x rate) + PSUM->SBUF copy on Act.
        # Terms: (tok8 + r8) @ wq8  (5 pairs over the stacked tokens), then
        # tok8 @ wq_lo8 (3 pairs; the last pairs r8_k0 against a zero
        # weight tile). Compensated fp8: quantization error ~0.1%. ----
        qts = {}

        def emit_qproj_mtile(c, m):
            if m == 0:
                qts[c] = qt_pool.tile([P, MT_Q, TCH], BF16, tag="qt",
                                      name="qt_sb")
            ps_q = ps_qo.tile([P, TCH], F32, tag="ps_qo", name="ps_q")
            mc = slice(m * P, (m + 1) * P)
            for k in range(KT_H):
                nc.tensor.matmul(
                    ps_q, wq_sb[:, k, mc],
                    toks[c][:, k, :], start=(k == 0), stop=(k == KT_H - 1))
            nc.scalar.copy(qts[c][:, m, :], ps_q)

        # ---- output projection subtile: 12 PE matmuls, PSUM->SBUF drains
        # split across Act/DVE (no bias add: bo rides Wo row 80), 1 DMA ----
        def emit_outproj_st(ctx_v, c, st, tail=False):
            tok_cols = ctx_v[:, :, st * P:(st + 1) * P]
            po1 = ps_qo.tile([P, NO1], F32, tag="ps_qo", name="po1")
            for k in range(MT_O):
                nc.tensor.matmul(po1, tok_cols[:, k, :], wo_sb[:, k, 0:NO1],
                                 start=(k == 0), stop=(k == MT_O - 1))
            po2 = ps_qo.tile([P, HID - NO1], F32, tag="ps_qo", name="po2")
            for k in range(MT_O):
                nc.tensor.matmul(po2, tok_cols[:, k, :], wo_sb[:, k, NO1:HID],
                                 start=(k == 0), stop=(k == MT_O - 1))
            out_sb = out_pool.tile([P, HID], F32)
            nc.scalar.copy(out_sb[:, 0:NO1], po1)
            # in the drain tail run the two PSUM copies on different
            # engines so they overlap
            (nc.vector.tensor_copy if tail else nc.scalar.copy)(
                out_sb[:, NO1:HID], po2)
            t0 = c * TCH + st * P
            nc.sync.dma_start(out=out[t0:t0 + P, :], in_=out_sb)

        # ---- K^T [EMB, S] as [128, 5, S], one parity-masked copy each ----
        # Emitted after the chunk-0 Q projection in PE order; only needs
        # wke/wko/ctx which stream in behind wq/tok0.
        def emit_kt(wk_sb, name):
            kt = consts.tile([P, MT_Q, S], BF16, name=name)
            for m in range(MT_Q):
                ps_k = ps_s.tile([P, S], F32, tag="ps_s", name="ps_k")
                for k in range(KT_C):
                    nc.tensor.matmul(
                        ps_k, wk_sb[:, k, m * P:(m + 1) * P], ctx_sb[:, k, :],
                        start=(k == 0), stop=(k == KT_C - 1))
                nc.vector.tensor_copy(kt[:, m, :], ps_k)
            return kt

        # ---- V [S, H, 128], with each head's columns ROTATED by the
        # head's ctx_v phase phi_h = (96h mod 128): V dim d sits at column
        # (phi_h + d) % 128. The attn@V output row (96h+off) % 128 then
        # equals ctx_v row 96h+off, so every drain piece has read base ==
        # write base (14 pieces per chunk). Head 0's col 80 is ones: the
        # normalized attn sums to ~1 there, which multiplies Wo row 80 =
        # bo (host-folded bias).
        def emit_v():
            v = consts.tile([S, H, P], BF16, name="v_sb")
            nc.vector.memset(v, 0.0)
            nc.vector.memset(v[:, 0, DH:DH + 1], 1.0)
            # all heads' V in two wide matmul groups (N=512/128) instead
            # of 8 narrow ones - 12 PE ops instead of 48 on the startup
            # critical path; copies scatter each head's rotated pieces
            for (n0, n1) in ((0, NO1), (NO1, EMB)):
                ps_v = ps_s.tile([S, n1 - n0], F32, tag="ps_s", name="ps_v")
                for k in range(KT_C):
                    nc.tensor.matmul(ps_v, ctx_sb[:, k, :],
                                     wv_sb[:, k, n0:n1],
                                     start=(k == 0), stop=(k == KT_C - 1))
                for h in range(H):
                    phi = (DHP * h) % P
                    base = h * DH
                    wrap = P - phi  # dims [0,wrap) at col phi+d, rest at d-wrap
                    for (d0, d1) in ((0, min(wrap, DH)), (min(wrap, DH), DH)):
                        g0 = max(base + d0, n0)
                        g1 = min(base + d1, n1)
                        if g0 >= g1:
                            continue
                        dd = g0 - base
                        col = phi + dd if dd < wrap else dd - wrap
                        nc.vector.tensor_copy(
                            v[:, h, col:col + (g1 - g0)],
                            ps_v[:, g0 - n0:g1 - n0])
            return v

        def emit_scores(c, h):
            kt = kt_ev if h % 2 == 0 else kt_od
            segs = SC_SEGS[h]
            ps_sc = ps_s.tile([S, TCH], F32, tag="ps_s", name="ps_sc")
            for i, (m, a, b) in enumerate(segs):
                nc.tensor.matmul(
                    ps_sc, kt[a:b, m, :], qts[c][a:b, m, :],
                    start=(i == 0), stop=(i == len(segs) - 1),
                    tile_position=(a, 0))
            return ps_sc

        # ---- chunk-0 prologue (interleaved with the DMA arrival order:
        # wqs/tok0, wke/ctx, tok1, wko, wv) ----
        for m in range(MT_Q):
            emit_qproj_mtile(0, m)
        kt_ev = emit_kt(wke_sb, "kt_ev")
        kt_od = emit_kt(wko_sb, "kt_od")
        v_sb = emit_v()

        prev_ctx = None  # ctx_v of the previous chunk (outproj deferred)
        for c in range(NCH):
            if c + 2 < NCH:
                toks[c + 2] = tok_pool.tile([P, KT_H, TCH], BF16, tag="tok",
                                            name="tok_sb")
                nc.sync.dma_start(
                    out=toks[c + 2],
                    in_=tokT_r[:, :, (c + 2) * TCH:(c + 3) * TCH])

            # exp into the head's at slot, then the denominator via Pool
            # all-reduce (SBUF only), reciprocal on DVE, and an IN-PLACE
            # bf16 normalize of the attention weights (so attn@V consumes
            # already-normalized weights and the PSUM drain is a plain copy)
            def emit_chain(c, h, ps_sc):
                slot = h % NSLOT
                nc.scalar.activation(
                    at_sb[0:S, slot, :], ps_sc,
                    mybir.ActivationFunctionType.Exp,
                    scale=float(SCALE))
                ar_sb = r_pool.tile([P, TCH], F32, tag="ar", name="ar_sb")
                nc.gpsimd.partition_all_reduce(
                    ar_sb, at_sb[:, slot, :], channels=P,
                    reduce_op=bass_isa.ReduceOp.add)
                rb_sb = r_pool.tile([P, TCH], F32, tag="rb", name="rb_sb")
                # full 128 partitions: the all-reduce broadcast the sum to
                # every partition, so no zero-divide; DVE ops keep the
                # 32-multiple partition counts the hardware likes
                nc.vector.reciprocal_approx_fast(out=rb_sb, in_=ar_sb)
                ats[h] = rb_sb

            # scores run LS slots ahead, the softmax chain LE slots ahead
            # of attn@V so the PE never blocks on the chain
            le = LE
            score_ps = {}
            ats = {}
            for j in range(min(LS, H)):
                score_ps[j] = emit_scores(c, j)
            for j in range(min(le, H)):
                emit_chain(c, j, score_ps.pop(j))
            ctx_v = ctxv_pool.tile([P, MT_O, TCH], BF16)
            drain_i = 0
            for h in range(H):
                if h + LS < H:
                    score_ps[h + LS] = emit_scores(c, h + LS)
                if h + le < H:
                    emit_chain(c, h + le, score_ps.pop(h + le))

                # interleaved deferred outproj + next-chunk Q projection:
                # keeps the PE busy while the softmax chain runs ahead.
                # The LAST chunk's Q projection is split: m0-m2 during the
                # previous chunk (feeding the score lookahead), m3-m4 in
                # its own early slots - evens PE load so neither phase is
                # purely chain-paced.
                if h % 2 == 1 and prev_ctx is not None:
                    emit_outproj_st(prev_ctx, c - 1, (h - 1) // 2)
                # chunk 0 additionally pulls qproj(2) m0-m2 into its empty
                # late slots (it has no outproj filler), chunk 1 emits the
                # remainder - same load-evening as at the tail
                if c == 0 and NCH > 2 and h >= 5:
                    emit_qproj_mtile(2, h - 5)
                if c + 1 < NCH:
                    if c == 1 and NCH > 2:
                        if h < 2:
                            emit_qproj_mtile(2, 3 + h)
                        elif False:
                            pass
                    else:
                        mlim = 3 if c + 1 == NCH - 1 else MT_Q
                        if h < mlim:
                            emit_qproj_mtile(c + 1, h)
                if c + 1 == NCH and h < MT_Q - 3:
                    emit_qproj_mtile(c, 3 + h)

                # ctx_aug^T [128, TCH] in the head's rotated row phase;
                # the drain normalizes on DVE (read base == write base
                # thanks to the V rotation; rb was computed off-chain)
                rb_sb = ats.pop(h)
                ps_c = ps_cv.tile([P, TCH], F32, tag="ps_cv", name="ps_c")
                nc.tensor.matmul(ps_c, v_sb[:, h, :],
                                 at_sb[0:S, h % NSLOT, :],
                                 start=True, stop=True)
                for (m, a, b) in _matmul_segments(h * DHP, DHP):
                    nc.vector.tensor_mul(
                        ctx_v[a:b, m, :], ps_c[a:b, :], rb_sb[a:b, :])
                    drain_i += 1

            prev_ctx = ctx_v

        for st in range(TCH // P):
            emit_outproj_st(prev_ctx, NCH - 1, st, tail=True)

    nc.compile()
    return nc


_PROGRAM = None


def _get_program():
    global _PROGRAM
    if _PROGRAM is None:
        _PROGRAM = _build_program()
    return _PROGRAM


BF16_NP = ml_dtypes.bfloat16


def _pad_head_rows(w, dtype=np.float32):
    """[H*DH, cols] -> [H*DHP, cols] zero-padded per head."""
    wp = np.zeros((EMBP, w.shape[1]), dtype)
    for h in range(H):
        wp[h * DHP:h * DHP + DH] = w[h * DH:(h + 1) * DH]
    return wp


def _parity_mask(w, parity):
    """Zero the columns of heads whose index parity != parity."""
    wm = np.array(w, np.float32, copy=True)
    for h in range(H):
        if h % 2 != parity:
            wm[:, h * DH:(h + 1) * DH] = 0.0
    return wm


F8_NP = ml_dtypes.float8_e4m3


def _fp8_stack_tokens(tokT):
    """[640, T] f32 -> [1280, T] fp8: [t0..t3, r0..r3, t4, r4] k-tiles
    where t = fp8(x) and r = fp8(x - fp8(x))."""
    t8 = tokT.astype(F8_NP)
    r8 = (tokT - t8.astype(np.float32)).astype(F8_NP)
    return np.concatenate([t8[0:512], r8[0:512], t8[512:640], r8[512:640]],
                          axis=0)


def _fp8_stack_wq(Wq):
    """[640, 640] f32 -> [1536, 640] fp8 k-stack [w0..w4, w4, wl0..wl4, Z]
    for the 3-term Qproj (w = fp8(32 Wq), wl = fp8 residual)."""
    Wp = WQ_SCALE_NP * Wq
    w8 = Wp.astype(F8_NP)
    wlo = (Wp - w8.astype(np.float32)).astype(F8_NP)
    z = np.zeros((P, Wq.shape[1]), F8_NP)
    return np.concatenate([w8, w8[512:640], wlo, z], axis=0)


WQ_SCALE_NP = np.float32(32.0)


def _prepare_in_maps(tokens, context, Wq, Wk, Wv, Wo, bo):
    tokens = np.asarray(tokens, np.float32)
    context = np.asarray(context, np.float32)
    wq_ = np.ascontiguousarray(np.asarray(Wq, np.float32)).astype(BF16_NP)
    wk_f = np.asarray(Wk, np.float32)
    wke_ = _parity_mask(wk_f, 0).astype(BF16_NP)
    wko_ = _parity_mask(wk_f, 1).astype(BF16_NP)
    wv_ = np.ascontiguousarray(np.asarray(Wv, np.float32)).astype(BF16_NP)
    wo_ = _pad_head_rows(np.asarray(Wo, np.float32))
    wo_[DH] = np.asarray(bo, np.float32)   # bias rides Wo pad row 80
    wo_ = wo_.astype(BF16_NP)
    in_maps = []
    for b in range(B):
        in_maps.append({
            "tokT": np.ascontiguousarray(tokens[b].T).astype(BF16_NP),
            "ctxT": np.ascontiguousarray(context[b].T).astype(BF16_NP),
            "wq": wq_, "wke": wke_, "wko": wko_, "wv": wv_, "wo": wo_,
        })
    return in_maps


def kernel(tokens, context, Wq, Wk, Wv, Wo, bo):
    nc = _get_program()
    in_maps = _prepare_in_maps(tokens, context, Wq, Wk, Wv, Wo, bo)
    res = bass_utils.run_bass_kernel_spmd(nc, in_maps, core_ids=list(range(B)))
    return np.stack([res.results[b]["out"] for b in range(B)])
